# revision 34
# baseline (speedup 1.0000x reference)
"""Trainium2 Bass kernel: ternary-conv BasicBlock (conv3x3 -> BN -> ReLU -> conv3x3 -> BN -> +res -> ReLU).

Sharding: data-parallel over batch across 8 NeuronCores (2 images/core),
conv weights replicated, BN batch stats exact via cross-core AllReduces.

The "noised" 1x1 extra term in the reference uses the centre tap of the same
ternary kernel; conv is linear in the weights, so it folds into the 3x3 kernel
by doubling the centre tap (done host-side during weight packing).

Conv padding: width is physically padded to 58 with zero columns so the PSUM
output stays contiguous per tap; height padding is virtual - the centre tap
runs first with start=True covering the full PSUM block and row-edge taps
accumulate into clipped row windows (PSUM has_written bits make partial-region
accumulation safe).

Schedule: both convs iterate output-channel-tile (co) major. Each co's BN
stats AllReduce is issued as soon as that co's conv tiles finish, so co0's
collective (and its BN apply / final elementwise) overlaps the co1 matmuls;
only co1's small warm AllReduce (~10us) sits on the critical path. Matmul
operands are bf16: the ternary weights are exact in bf16 and x rounding costs
~2.5e-3 rel err; f32r measured 2x slower (lowers to fp32_mode=HIGH).
"""
import numpy as np
import ml_dtypes

import concourse.bass as bass
import concourse.bacc as bacc
import concourse.tile as tile
import concourse.mybir as mybir
from concourse import bass_utils

NCORES = 8
NI = 2              # images per core (batch 16 / 8 cores)
C = 256
P = 128
CT = C // P         # channel tiles of 128
H = W = 56
WP = 58             # physically padded width (zero cols 0 and 57)
BR = 8              # output rows per PSUM block
NB = H // BR        # 7 blocks per image
NTOT = 16 * H * W   # BN divisor over the full batch
BN_EPS = 1e-5

F32 = mybir.dt.float32
AF = mybir.ActivationFunctionType
ALU = mybir.AluOpType
AX = mybir.AxisListType

# bf16 streams 1 col/cycle with FWL weight loads; ternary weights are exact
MM_DTYPE = "bf16"   # "f32r" | "bf16"

# taps with the full-coverage centre tap first (start=True zeroes the block)
TAPS = [(1, 1)] + [(ky, kx) for ky in range(3) for kx in range(3) if (ky, kx) != (1, 1)]


def _mm_dt():
    return mybir.dt.float32r if MM_DTYPE == "f32r" else mybir.dt.bfloat16


def build(collective=True, num_devices=NCORES):
    DT = _mm_dt()
    nc = bacc.Bacc("TRN2", target_bir_lowering=False, debug=False,
                   num_devices=num_devices)
    x_d = nc.dram_tensor("x", [NI, C, H, W], F32, kind="ExternalInput")
    w_d = nc.dram_tensor("wq", [2, P, 36 * P], DT, kind="ExternalInput")
    gb_d = nc.dram_tensor("gb", [P, 8], F32, kind="ExternalInput")
    out_d = nc.dram_tensor("out", [NI, C, H, W], F32, kind="ExternalOutput")

    with tile.TileContext(nc) as tc:
        with (
            tc.tile_pool(name="const", bufs=1) as constp,
            tc.tile_pool(name="wpool", bufs=2) as wpool,
            tc.tile_pool(name="data", bufs=1) as datap,
            tc.tile_pool(name="small", bufs=1) as smallp,
            tc.tile_pool(name="psum", bufs=8, space=bass.MemorySpace.PSUM) as psump,
            tc.tile_pool(name="dram", bufs=1, space="DRAM") as dramp,
        ):
            # conv1 centre-tap weight groups first: they gate the first
            # matmul. Weights ride ScalarE's DMA queue so they issue in
            # parallel with the x staging on the sync queue.
            wsb0 = wpool.tile([P, 36, P], DT, tag="w", name="wsb0")
            wsrc0 = w_d[0].rearrange("p (g m) -> p g m", g=36)
            nc.scalar.dma_start(wsb0[:, 16:20, :], wsrc0[:, 16:20, :])
            if MM_DTYPE == "f32r":
                nc.scalar.copy(wsb0[:, 16:20, :], wsb0[:, 16:20, :])

            # x staging in f32; the slots are reused later for conv1 raw outputs
            xs = [datap.tile([P, NI, H, W], F32, tag=f"raw{t}", name=f"xs{t}")
                  for t in range(CT)]
            HH = H // 2
            # finer pieces keep the staging cast pipelined ahead of the conv
            # row consumption; piece 0 stays 9 rows (what conv block 0 needs)
            ROWPIECES = {0: ((0, 9), (9, 20), (20, 32), (32, 44), (44, H)),
                         1: ((0, 14), (14, HH), (HH, 42), (42, H))}
            # first 9 rows of image 0 ahead of the bulk weight DMA
            for t in range(CT):
                nc.sync.dma_start(xs[t][:, 0, 0:9], x_d[0, t * P:(t + 1) * P, 0:9])

            # conv inputs rounded to the matmul dtype, width-padded to 58
            xr = [datap.tile([P, NI * H * WP + 2], DT, tag=f"xr{t}",
                             name=f"xr{t}") for t in range(CT)]
            h1 = [datap.tile([P, NI * H * WP + 2], DT, tag=f"h1{t}",
                             name=f"h1_{t}") for t in range(CT)]

            def _img(buf, t, i):
                # [P, H, WP] view of image i inside the flat padded tile
                return buf[t][:, i * H * WP:(i + 1) * H * WP].rearrange(
                    "p (h w) -> p h w", h=H)

            # HAM pre-warm: throwaway matmuls during the staging dead time
            # push the PE clock gate toward 8/8 before the real stream; the
            # burst is short enough to retire before piece 0 is staged
            warm = constp.tile([P, 64], DT)
            nc.vector.memset(warm[:], 0.0)
            wpt = psump.tile([P, 64], F32, tag="acc")
            for _ in range(36):
                nc.tensor.matmul(wpt[0:64, :], warm[:, 0:64], warm[:],
                                 start=True, stop=True)

            # piece-0 cast issues on ScalarE before the bulk-weight DMA
            # issue ops so the first matmul isn't queued behind them
            for t in range(CT):
                nc.scalar.activation(_img(xr, t, 0)[:, 0:9, 1:57],
                                     xs[t][:, 0, 0:9], AF.Identity)
            # bulk of conv1 weights, split so each tap's groups arrive
            # just-in-time behind the centre tap (TAPS order is (0,0),(0,1),..)
            for lo, hi in ((0, 4), (4, 8), (8, 16), (20, 28), (28, 36)):
                nc.scalar.dma_start(wsb0[:, lo:hi, :], wsrc0[:, lo:hi, :])
                if MM_DTYPE == "f32r":
                    nc.scalar.copy(wsb0[:, lo:hi, :], wsb0[:, lo:hi, :])

            gbsb = constp.tile([P, 8], F32)
            nc.scalar.dma_start(gbsb[:], gb_d[:])
            for i in range(NI):
                for r0, r1 in ROWPIECES[i]:
                    if (i, r0) == (0, 0):
                        continue   # already issued ahead of the bulk weights
                    for t in range(CT):
                        nc.sync.dma_start(
                            xs[t][:, i, r0:r1],
                            x_d[i, t * P:(t + 1) * P, r0:r1])

            zcol = constp.tile([P, H], F32)
            nc.vector.memset(zcol[:], 0.0)
            epsc = constp.tile([P, 1], F32)
            nc.vector.memset(epsc[:], BN_EPS)
            for i in range(NI):
                for t in range(CT):
                    for c in (0, WP - 1):
                        nc.vector.tensor_copy(_img(xr, t, i)[:, :, c], zcol[:])
                        nc.vector.tensor_copy(_img(h1, t, i)[:, :, c], zcol[:])
                for r0, r1 in ROWPIECES[i]:
                    if (i, r0) == (0, 0):
                        continue   # piece-0 cast already issued above
                    for t in range(CT):
                        # f32 -> bf16 rounding copy on ScalarE (idle until the
                        # first BN chain), keeping the DVE free for evictions
                        nc.scalar.activation(
                            _img(xr, t, i)[:, r0:r1, 1:57],
                            xs[t][:, i, r0:r1], AF.Identity)

            def _taps_mm(pt, wsb, srcs, ci, co, i, h0, k0, klast):
                k = k0
                for ky, kx in TAPS:
                    # valid output rows for this tap (height pad is virtual)
                    hs = max(h0, 1 - ky)
                    he = min(h0 + BR - 1, H - ky)
                    nr = he - hs + 1
                    ri = hs + ky - 1
                    g = ((ky * 3 + kx) * CT + ci) * CT + co
                    rhs = _img(srcs, ci, i)[:, ri:ri + nr, kx:kx + W]
                    outp = pt[:, hs - h0:hs - h0 + nr, :]
                    nc.tensor.matmul(outp, wsb[:, g, :], rhs,
                                     start=(k == 0), stop=(k == klast))
                    k += 1

            def conv_co(conv, co, wsb, srcs, raws):
                """All matmul tiles of one output-channel tile: returns the
                per-block channel sums / sums-of-squares [P, NI*NB].

                The whole stats trigger path (evict, square, reduce) lives on
                the DVE so no collective-gated consumer (ScalarE/GpSimd) can
                head-of-line block it in a strict-FIFO queue."""
                part_sum = smallp.tile([P, NI * NB], F32, tag=f"ps{conv}{co}",
                                       name=f"psum{conv}{co}")
                part_sq = smallp.tile([P, NI * NB], F32, tag=f"pq{conv}{co}",
                                      name=f"psq{conv}{co}")
                for i in range(NI):
                    for blk in range(NB):
                        h0 = blk * BR
                        pt = psump.tile([P, BR, W], F32, tag="acc")
                        for ci in range(CT):
                            _taps_mm(pt, wsb, srcs, ci, co, i, h0, 9 * ci, 17)
                        r = i * NB + blk
                        raw = raws[co][:, i, h0:h0 + BR, :]
                        # evict PSUM -> SBUF f32, accumulating the channel sum
                        nc.vector.tensor_scalar(
                            raw, pt[:], 0.0, 0.0, ALU.bypass, ALU.add,
                            accum_out=part_sum[:, r:r + 1])
                        # sum-of-squares as pt*raw on DVE (one PSUM read; a
                        # dual read of the same SBUF range hangs the DVE), so
                        # ScalarE stays free for the AllReduce-gated BN chain
                        nc.vector.scalar_tensor_tensor(
                            pt[:], pt[:], 1.0, raw, ALU.mult, ALU.mult,
                            accum_out=part_sq[:, r:r + 1])
                return part_sum, part_sq

            def conv_co_partial(co, wsb, srcs, raws):
                """conv2 pass A: ci0's taps only, partials evicted to
                raws[co]. Reads only h1[0], so the PE can flow straight from
                conv1 into this work while BN1-co1's AllReduce is in flight."""
                for i in range(NI):
                    for blk in range(NB):
                        h0 = blk * BR
                        pt = psump.tile([P, BR, W], F32, tag="acc")
                        _taps_mm(pt, wsb, srcs, 0, co, i, h0, 0, 8)
                        nc.vector.tensor_copy(
                            raws[co][:, i, h0:h0 + BR, :], pt[:])

            def conv_co_merge(co, wsb, srcs, raws):
                """conv2 pass B: ci1's taps, merged with pass A's partials;
                stats accumulate off the merged result."""
                part_sum = smallp.tile([P, NI * NB], F32, tag=f"ps1{co}",
                                       name=f"psum1{co}")
                part_sq = smallp.tile([P, NI * NB], F32, tag=f"pq1{co}",
                                      name=f"psq1{co}")
                for i in range(NI):
                    for blk in range(NB):
                        h0 = blk * BR
                        pt = psump.tile([P, BR, W], F32, tag="acc")
                        _taps_mm(pt, wsb, srcs, 1, co, i, h0, 0, 8)
                        r = i * NB + blk
                        raw = raws[co][:, i, h0:h0 + BR, :]
                        nc.vector.scalar_tensor_tensor(
                            raw, pt[:], 1.0, raw, ALU.mult, ALU.add,
                            accum_out=part_sum[:, r:r + 1])
                        # square of the merged result; the (dead) PSUM block
                        # is the scratch output
                        nc.scalar.activation(pt[:], raw, AF.Square,
                                             accum_out=part_sq[:, r:r + 1])
                return part_sum, part_sq

            def stats_start(conv, co, part_sum, part_sq):
                """Reduce this co's partials and launch its AllReduce."""
                stats = smallp.tile([P, 2], F32, tag=f"st{conv}{co}")
                nc.vector.reduce_sum(stats[:, 0:1], part_sum[:], axis=AX.X)
                nc.vector.reduce_sum(stats[:, 1:2], part_sq[:], axis=AX.X)
                if not collective:
                    return stats
                b_in = dramp.tile([P, 2], F32, tag=f"bi{conv}{co}")
                b_out = dramp.tile([P, 2], F32, tag=f"bo{conv}{co}")
                # HWDGE (sync queue) completion semaphores post ~6us faster
                # than the gpsimd SWDGE path, so the collective doorbell
                # (which waits on this dma) fires that much earlier
                nc.sync.dma_start(b_in[:], stats[:])
                nc.gpsimd.collective_compute(
                    "AllReduce", ALU.add,
                    replica_groups=[list(range(num_devices))],
                    ins=[b_in.opt()], outs=[b_out.opt()])
                return b_out

            def bn_params(conv, co, red):
                """Finish BN from the (reduced) stats: scale/shift [P,1].

                ScalarE-only chain (out = func(in*scale + bias)) so the
                AllReduce-gated ops never sit in the DVE queue, which carries
                the next collective's trigger path (strict FIFO queues)."""
                if collective:
                    gstats = smallp.tile([P, 2], F32, tag=f"gst{conv}{co}")
                    # ScalarE's own DMA queue: lands right before the smalls
                    # chain that consumes it, and never queues behind the
                    # b_in trigger DMAs (sync) or the eviction stream (DVE)
                    nc.scalar.dma_start(gstats[:], red[:])
                else:
                    gstats = red
                inv_n = 1.0 / NTOT
                m2 = smallp.tile([P, 1], F32, tag=f"m2{conv}{co}")
                msq = smallp.tile([P, 1], F32, tag=f"msq{conv}{co}")
                var = smallp.tile([P, 1], F32, tag=f"var{conv}{co}")
                rstd = smallp.tile([P, 1], F32, tag=f"rstd{conv}{co}")
                scl = smallp.tile([P, 1], F32, tag=f"scl{conv}{co}")
                tmp = smallp.tile([P, 1], F32, tag=f"tmp{conv}{co}")
                sft = smallp.tile([P, 1], F32, tag=f"sft{conv}{co}")
                g_ap = gbsb[:, conv * 4 + co: conv * 4 + co + 1]
                b_ap = gbsb[:, conv * 4 + 2 + co: conv * 4 + 3 + co]
                std = smallp.tile([P, 1], F32, tag=f"std{conv}{co}")
                nc.scalar.activation(m2[:], gstats[:, 0:1], AF.Square, scale=inv_n)
                nc.scalar.activation(msq[:], gstats[:, 1:2], AF.Identity,
                                     scale=inv_n)
                nc.scalar.activation(var[:], m2[:], AF.Identity, scale=-1.0,
                                     bias=msq[:])
                # Sqrt shares ScalarE's resident table (Ln/Exp would force two
                # ~1.3us table swaps per call); the reciprocal is the one DVE
                # op in the chain — each call site places it where it cannot
                # head-of-line block the eviction/reduce stream
                nc.scalar.activation(std[:], var[:], AF.Sqrt, bias=epsc[:])
                nc.vector.reciprocal(rstd[:], std[:])
                nc.scalar.activation(scl[:], rstd[:], AF.Identity, scale=g_ap)
                nc.scalar.activation(tmp[:], gstats[:, 0:1], AF.Identity,
                                     scale=scl[:])
                nc.scalar.activation(sft[:], tmp[:], AF.Identity, scale=-inv_n,
                                     bias=b_ap)
                return scl, sft

            def bn1_apply(co, scl, sft, raws1):
                # fused scale/shift/relu/round-to-DT on ScalarE into h1[co]
                for i in range(NI):
                    for r0, r1 in ROWPIECES[i]:
                        dst = _img(h1, co, i)[:, r0:r1, 1:57]
                        srcv = raws1[co][:, i, r0:r1]
                        nc.scalar.activation(dst, srcv, AF.Relu,
                                             bias=sft[:], scale=scl[:])

            QH = H // 4

            def finish_out(co, scl, sft, raws2):
                # out = relu(h1 + scl*raw2 + sft), in place in raws2[co]
                for i in range(NI):
                    for qq in range(4):
                        rs = slice(qq * QH, (qq + 1) * QH)
                        v = raws2[co][:, i, rs]
                        h1v = _img(h1, co, i)[:, rs, 1:57]
                        if MM_DTYPE == "f32r":
                            h1v = h1v.bitcast(F32)
                        nc.vector.scalar_tensor_tensor(
                            v, v, scl[:], h1v, ALU.mult, ALU.add)
                        nc.scalar.activation(v, v, AF.Relu, bias=sft[:])
                        nc.sync.dma_start(out_d[i, co * P:(co + 1) * P, rs], v)

            # ---- conv1, co-major with per-co overlapped BN collectives ----
            raws1 = [datap.tile([P, NI, H, W], F32, tag=f"raw{t}",
                                name=f"raws1_{t}") for t in range(CT)]
            ps10, pq10 = conv_co(0, 0, wsb0, xr, raws1)
            red10 = stats_start(0, 0, ps10, pq10)   # AR(co0) hides under co1

            # conv2 weights DMA overlaps conv1-co1 matmuls
            wsb1 = wpool.tile([P, 36, P], DT, tag="w", name="wsb1")
            nc.sync.dma_start(wsb1[:], w_d[1].rearrange("p (g m) -> p g m", g=36))
            if MM_DTYPE == "f32r":
                nc.scalar.copy(wsb1[:], wsb1[:])

            ps11, pq11 = conv_co(0, 1, wsb0, xr, raws1)
            # co1's stats reduce + AllReduce trigger are emitted BEFORE the
            # fence; the scheduler-only fence then pins every AR-gated op
            # after the whole trigger path, so none of them can head-of-line
            # block a strict-FIFO engine queue when the collective is late
            red11 = stats_start(0, 1, ps11, pq11)
            tc.no_sync_barrier()
            scl10, sft10 = bn_params(0, 0, red10)
            bn1_apply(0, scl10, sft10, raws1)

            # ---- conv2 in two passes; raws2 reuses the xr slots ----
            # pass A (ci0-only, both co tiles) reads just h1[0], so ~52us of
            # matmul flows with no dependence on BN1-co1's AllReduce: the
            # collective latency (and any cross-core launch skew) hides here.
            # A-co0 is emitted between the two BN1 finishes so its DVE evicts
            # sit between recip10 and recip11 in the queue — each reciprocal
            # is ready before the evicts that follow it are needed.
            raws2 = [datap.tile([P, NI, H, W], F32, tag=f"xr{t}",
                                name=f"raws2_{t}") for t in range(CT)]
            conv_co_partial(0, wsb1, h1, raws2)
            scl11, sft11 = bn_params(0, 1, red11)
            bn1_apply(1, scl11, sft11, raws1)
            conv_co_partial(1, wsb1, h1, raws2)
            ps20, pq20 = conv_co_merge(0, wsb1, h1, raws2)
            red20 = stats_start(1, 0, ps20, pq20)   # AR hides under B-co1
            ps21, pq21 = conv_co_merge(1, wsb1, h1, raws2)
            red21 = stats_start(1, 1, ps21, pq21)
            # two fences: co0's whole finish (smalls, stt, relu, out-DMA)
            # orders before co1's in every queue, so it streams out the moment
            # its data is ready instead of queueing behind AR(co1)-gated ops
            tc.no_sync_barrier()
            scl20, sft20 = bn_params(1, 0, red20)
            finish_out(0, scl20, sft20, raws2)
            tc.no_sync_barrier()
            scl21, sft21 = bn_params(1, 1, red21)
            finish_out(1, scl21, sft21, raws2)

    nc.compile()
    return nc


def _quantize(w):
    """Ternary quantization matching reference.noised_tri_conv, on jax CPU,
    with the centre tap doubled (folds the 'noised' 1x1 einsum term)."""
    try:
        import jax
        import jax.numpy as jnp
        cpu = jax.devices("cpu")[0]
        with jax.default_device(cpu):
            wj = jnp.asarray(np.asarray(w, np.float32))
            tw = wj - jnp.mean(wj)
            mx, mn = jnp.max(tw), jnp.min(tw)
            lo = mn + (mx - mn) / 3
            hi = mx - (mx - mn) / 3
            tq = jnp.where(tw < lo, -1.0,
                           jnp.where(tw > hi, 1.0, 0.0)).astype(wj.dtype)
            tq = np.asarray(tq).copy()
    except Exception:
        wf = np.asarray(w, np.float32)
        tw = (wf - np.float32(wf.mean(dtype=np.float32))).astype(np.float32)
        mx, mn = np.float32(tw.max()), np.float32(tw.min())
        lo = np.float32(mn + (mx - mn) / np.float32(3))
        hi = np.float32(mx - (mx - mn) / np.float32(3))
        tq = np.where(tw < lo, np.float32(-1.0),
                      np.where(tw > hi, np.float32(1.0), np.float32(0.0)))
        tq = tq.astype(np.float32)
    tq[:, :, 1, 1] *= 2.0
    return tq


def _pack_weights(w1, w2):
    np_dt = np.float32 if MM_DTYPE == "f32r" else ml_dtypes.bfloat16
    wq = np.zeros((2, P, 36 * P), np_dt)
    for conv, w in enumerate((w1, w2)):
        q = _quantize(w)                      # [O=256, I=256, 3, 3]
        q6 = q.reshape(CT, P, CT, P, 3, 3)    # [co_t, pco, ci_t, pci, ky, kx]
        for ky in range(3):
            for kx in range(3):
                for ci in range(CT):
                    for co in range(CT):
                        g = ((ky * 3 + kx) * CT + ci) * CT + co
                        wq[conv, :, g * P:(g + 1) * P] = \
                            q6[co, :, ci, :, ky, kx].T.astype(np_dt)
    return wq


def _pack_gb(g1, b1, g2, b2):
    gb = np.zeros((P, 8), np.float32)
    for conv, (g, b) in enumerate(((g1, b1), (g2, b2))):
        for t in range(CT):
            gb[:, conv * 4 + t] = np.asarray(g, np.float32)[t * P:(t + 1) * P]
            gb[:, conv * 4 + 2 + t] = np.asarray(b, np.float32)[t * P:(t + 1) * P]
    return gb


_CACHE = {}


def _get_nc():
    if "nc" not in _CACHE:
        _CACHE["nc"] = build()
    return _CACHE["nc"]


def make_in_maps(x, w1, w2, g1, b1, g2, b2):
    x = np.asarray(x, np.float32)
    wq = _pack_weights(w1, w2)
    gb = _pack_gb(g1, b1, g2, b2)
    return [{"x": np.ascontiguousarray(x[NI * c: NI * (c + 1)]),
             "wq": wq, "gb": gb} for c in range(NCORES)]


def kernel(x, w1, w2, g1, b1, g2, b2):
    nc = _get_nc()
    in_maps = make_in_maps(x, w1, w2, g1, b1, g2, b2)
    res = bass_utils.run_bass_kernel_spmd(nc, in_maps, core_ids=list(range(NCORES)))
    return np.concatenate([res.results[c]["out"] for c in range(NCORES)], axis=0)
